# revision 1
# baseline (speedup 1.0000x reference)
"""Trainium2 Bass kernel for nn_AttachmentPredictor.

Pipeline (per core, data-parallel over batch; 32 batches/core):
  x is pre-transposed on host to feature-major xT [D=1024, rows=32*256].
  stage1: head projection, feature-major psum[jt] += Wh[dk,jt] @ xT[dk, :]
  bias:   per-batch prep/child projections, feature-major [512, 32]
  tanh(Y1 + bias) -> c1, two 256-col segments per tile (per-batch bias)
  stage2/3: hidden layers, feature-major, tanh -> c2, c3
  scorer: [1,512] psum rows of scores via M=1 matmuls
  epilogue: reshape scores to [32, 256], exp(scores + logmask) with
  accumulated row sums, normalize, DMA out [32, 254].

Matmuls run as float32r (TF32-like, full PE rate) or bfloat16 per OPTS.
"""

import ml_dtypes
import numpy as np

import concourse.bass as bass
import concourse.mybir as mybir
import concourse.tile as tile
from concourse import bass_utils
from concourse.bass import ts

F32 = mybir.dt.float32
F32R = mybir.dt.float32r
BF16 = mybir.dt.bfloat16
AF = mybir.ActivationFunctionType

B, S, D, P = 256, 256, 1024, 512
NCORES = 8
BC = B // NCORES            # 32 batches per core
ROWS = BC * S               # 8192 rows per core
NBLK = ROWS // 512          # 16 blocks of 512 rows (2 batches each)
KD = D // 128               # 8 k-tiles over D
KP = P // 128               # 4 k-tiles over P
EPS = 1e-7
NEG = -1e9

OPTS = {
    "s1_dtype": "f32r",   # dtype for stage-1 x and Wh: "f32r" | "bf16"
    "mm_dtype": "f32r",   # dtype for stages 2/3, scorer, bias: "f32r" | "bf16"
    "group": 1,           # blocks processed jointly (lhsT back-to-back reuse)
    "xr_bufs": 4,
    "c_bufs": 10,
    "ps_bufs": 8,
}

_DT = {"f32r": F32R, "bf16": BF16, "f32": F32, "f16": mybir.dt.float16}
_NPDT = {"f32r": np.float32, "bf16": ml_dtypes.bfloat16, "f32": np.float32,
         "f16": np.float16}


# ---------------------------------------------------------------------------
# walrus in this container accepts at most ONE sync wait per instruction;
# split extra waits onto preceding NoOps on the same engine.
def _split_waits(nc, maxw=1):
    ctr = 0
    for f in nc.m.functions:
        for blk in f.blocks:
            insts = blk.instructions
            newlist = []
            changed = False
            for inst in insts:
                si = inst.sync_info
                if si is not None and len(si.on_wait) > maxw:
                    waits = list(si.on_wait)
                    keep = waits[len(waits) - maxw:]
                    extra = waits[: len(waits) - maxw]
                    for j in range(0, len(extra), maxw):
                        ctr += 1
                        newlist.append(
                            mybir.InstNoOp(
                                name=f"waitsplit-{ctr}",
                                engine=inst.engine,
                                ins=[],
                                outs=[],
                                sync_info=mybir.SyncInfo(
                                    on_wait=extra[j: j + maxw], on_update=[]
                                ),
                            )
                        )
                    inst.sync_info = mybir.SyncInfo(
                        on_wait=keep, on_update=list(si.on_update)
                    )
                    changed = True
                newlist.append(inst)
            if changed:
                insts[:] = newlist


# ---------------------------------------------------------------------------
def _build(opts=None, reps=1):
    opts = dict(OPTS, **(opts or {}))
    nc = bass.Bass("TRN2", target_bir_lowering=False, debug=False)

    S1DT = _DT[opts["s1_dtype"]]
    MMDT = _DT[opts["mm_dtype"]]
    G = opts["group"]
    assert NBLK % G == 0

    # All inputs arrive host-cast to the matmul dtypes (f32r shares the
    # f32 byte layout - the PE rounds on read), so plain HWDGE DMAs suffice.
    s1_dma = nc.sync
    mm_dma = nc.sync

    xT_d = nc.dram_tensor("xT", [NBLK, 128, KD * 512], S1DT,
                          kind="ExternalInput").ap()
    xp_d = nc.dram_tensor("xprep", [D, BC], S1DT, kind="ExternalInput").ap()
    xc_d = nc.dram_tensor("xchild", [D, BC], S1DT, kind="ExternalInput").ap()
    wh_d = nc.dram_tensor("wh", [D, P], S1DT, kind="ExternalInput").ap()
    wp_d = nc.dram_tensor("wp", [D, P], S1DT, kind="ExternalInput").ap()
    wc_d = nc.dram_tensor("wc", [D, P], S1DT, kind="ExternalInput").ap()
    w0_d = nc.dram_tensor("w0", [P, P], MMDT, kind="ExternalInput").ap()
    w1_d = nc.dram_tensor("w1", [P, P], MMDT, kind="ExternalInput").ap()
    sc_d = nc.dram_tensor("scT", [128, KP], MMDT, kind="ExternalInput").ap()
    lm_d = nc.dram_tensor("lmask", [BC, S], F32, kind="ExternalInput").ap()
    out_d = nc.dram_tensor("out", [BC, S - 2], F32, kind="ExternalOutput").ap()

    with tile.TileContext(nc) as tc:
        with (
            tc.tile_pool(name="consts", bufs=1) as consts,
            tc.tile_pool(name="ssb", bufs=3) as spool,
            tc.tile_pool(name="xr", bufs=opts["xr_bufs"]) as xpool,
            tc.tile_pool(name="acts", bufs=opts["c_bufs"]) as cpool,
            tc.tile_pool(name="ps", bufs=opts["ps_bufs"], space="PSUM") as pspool,
            tc.tile_pool(name="epi", bufs=1) as epi,
            tc.tile_pool(name="dram", bufs=1, space="DRAM") as dpool,
        ):
            # ---- constants -------------------------------------------------
            def load_packed(dram, k, n, dt, dma, tag):
                t = consts.tile([128, k * n], dt, tag=tag)
                dma.dma_start(
                    t[:].rearrange("p (k n) -> p k n", n=n),
                    dram.rearrange("(k p) n -> p k n", p=128),
                )
                return t

            if opts.get("marker"):
                mk = consts.tile([1, 4], F32, tag="marker")
                nc.gpsimd.memset(mk[:], float(opts["marker"]))
            wh_t = []
            for dk in range(KD):
                wt = consts.tile([128, P], S1DT, tag=f"wh{dk}", name=f"wh_t{dk}")
                s1_dma.dma_start(wt[:], wh_d[dk * 128: (dk + 1) * 128, :])
                wh_t.append(wt)
            xp_r = load_packed(xp_d, KD, BC, S1DT, mm_dma, "xp")
            xc_r = load_packed(xc_d, KD, BC, S1DT, mm_dma, "xc")
            sc_r = consts.tile([128, KP], MMDT, tag="sc")
            mm_dma.dma_start(sc_r[:], sc_d[:])

            # ---- per-batch bias, feature-major [128 j, 32 b] per j-tile ----
            # wp/wc are streamed one [128, 512] d-tile at a time.
            psbs = [pspool.tile([128, BC], F32, tag="ps", name=f"psb_{jt}")
                    for jt in range(KP)]
            for i, (xs, w_d) in enumerate(((xp_r, wp_d), (xc_r, wc_d))):
                for dk in range(KD):
                    wst = spool.tile([128, P], S1DT, tag="wst")
                    mm_dma.dma_start(wst[:], w_d[dk * 128: (dk + 1) * 128, :])
                    for jt in range(KP):
                        nc.tensor.matmul(
                            psbs[jt][:],
                            wst[:, jt * 128: (jt + 1) * 128],
                            xs[:, dk * BC: (dk + 1) * BC],
                            start=(i == 0 and dk == 0),
                            stop=(i == 1 and dk == KD - 1),
                        )
            bias_fm = []
            for jt in range(KP):
                bf = consts.tile([128, BC], F32, tag=f"bias{jt}")
                nc.vector.tensor_copy(bf[:], psbs[jt][:])
                bias_fm.append(bf)

            w0_r = load_packed(w0_d, KP, P, MMDT, mm_dma, "w0")
            w1_r = load_packed(w1_d, KP, P, MMDT, mm_dma, "w1")

            # ---- main loop: groups of G blocks (512 rows each) -------------
            for _rep in range(reps):
                for g in range(NBLK // G):
                    blks = [g * G + i for i in range(G)]
                    xrs = []
                    for blk in blks:
                        xr = xpool.tile([128, KD * 512], S1DT, tag="xr")
                        hw = KD * 512 // 2
                        for h in range(2):
                            s1_dma.dma_start(
                                xr[:, h * hw: (h + 1) * hw],
                                xT_d[blk, :, h * hw: (h + 1) * hw],
                            )
                        xrs.append(xr)

                    # stage 1
                    c1 = [[None] * KP for _ in blks]
                    for jt in range(KP):
                        pss1 = [pspool.tile([128, 512], F32, tag="ps", name=f"ps1_{g}_{jt}_{i}")
                                for i in range(G)]
                        for dk in range(KD):
                            for i in range(G):
                                nc.tensor.matmul(
                                    pss1[i][:],
                                    wh_t[dk][:, jt * 128: (jt + 1) * 128],
                                    xrs[i][:, dk * 512: (dk + 1) * 512],
                                    start=(dk == 0),
                                    stop=(dk == KD - 1),
                                )
                        for i, blk in enumerate(blks):
                            ct = cpool.tile([128, 512], MMDT, tag="c1")
                            for seg in range(2):
                                b = 2 * blk + seg
                                nc.scalar.activation(
                                    ct[:, ts(seg, 256)],
                                    pss1[i][:, ts(seg, 256)],
                                    AF.Tanh,
                                    bias=bias_fm[jt][:, b: b + 1],
                                )
                            c1[i][jt] = ct

                    # stages 2, 3
                    c_in = c1
                    stages23 = () if opts.get("skip_hidden") else ((2, w0_r), (3, w1_r))
                    for stage, w_r in stages23:
                        c_out = [[None] * KP for _ in blks]
                        for qt in range(KP):
                            pss2 = [pspool.tile([128, 512], F32, tag="ps", name=f"ps{stage}_{g}_{qt}_{i}")
                                    for i in range(G)]
                            for jk in range(KP):
                                for i in range(G):
                                    nc.tensor.matmul(
                                        pss2[i][:],
                                        w_r[:, jk * P + qt * 128:
                                            jk * P + (qt + 1) * 128],
                                        c_in[i][jk][:],
                                        start=(jk == 0),
                                        stop=(jk == KP - 1),
                                    )
                            for i in range(G):
                                ct = cpool.tile([128, 512], MMDT,
                                                tag=f"c{stage}")
                                nc.scalar.activation(ct[:], pss2[i][:], AF.Tanh)
                                c_out[i][qt] = ct
                        c_in = c_out

                    # scorer + block-local masked exp-normalization.
                    # The [1, 512] psum holds both batches along the free dim
                    # (cols b*256..), so per-batch [1, 256] slices stay at
                    # partition base 0 (32-alignment rule).
                    for i, blk in enumerate(blks):
                        pss = pspool.tile([1, 512], F32, tag="ps")
                        for qk in range(KP):
                            nc.tensor.matmul(
                                pss[:],
                                sc_r[:, qk: qk + 1],
                                c_in[i][qk][:],
                                start=(qk == 0),
                                stop=(qk == KP - 1),
                            )
                        for bi in range(2):
                            b = 2 * blk + bi
                            lmb = spool.tile([1, S], F32, tag="lmb",
                                             name=f"lmb_{g}_{i}_{bi}")
                            nc.sync.dma_start(lmb[:], lm_d[b: b + 1, :])
                            expin_b = spool.tile([1, S], F32, tag="expin_b",
                                                 name=f"ei_{g}_{i}_{bi}")
                            nc.vector.tensor_add(
                                expin_b[:], pss[0:1, bi * S: (bi + 1) * S],
                                lmb[:],
                            )
                            expm_b = spool.tile([1, S], F32, tag="expm_b",
                                                name=f"em_{g}_{i}_{bi}")
                            sums_b = spool.tile([1, 1], F32, tag="sums_b",
                                                name=f"su_{g}_{i}_{bi}")
                            nc.scalar.activation(expm_b[:], expin_b[:], AF.Exp,
                                                 accum_out=sums_b[:])
                            nc.vector.tensor_scalar_add(
                                sums_b[:], sums_b[:], EPS
                            )
                            recip_b = spool.tile([1, 1], F32, tag="recip_b",
                                                 name=f"re_{g}_{i}_{bi}")
                            nc.vector.reciprocal(recip_b[:], sums_b[:])
                            outv_b = spool.tile([1, S], F32, tag="outv_b",
                                                name=f"ov_{g}_{i}_{bi}")
                            nc.vector.tensor_scalar_mul(
                                outv_b[:], expm_b[:], recip_b[:]
                            )
                            nc.sync.dma_start(
                                out_d[b: b + 1, :], outv_b[:, 0: S - 2]
                            )


    _split_waits(nc)
    return nc


# ---------------------------------------------------------------------------
def _host_prep(x, proj_head, proj_prep, proj_child, hidden_layers, scorer, mask,
               opts=None):
    opts = dict(OPTS, **(opts or {}))
    s1_np = _NPDT[opts["s1_dtype"]]
    mm_np = _NPDT[opts["mm_dtype"]]
    x = np.asarray(x, np.float32)
    mask = np.asarray(mask)
    wh = np.ascontiguousarray(np.asarray(proj_head, s1_np))
    wp = np.ascontiguousarray(np.asarray(proj_prep, s1_np))
    wc = np.ascontiguousarray(np.asarray(proj_child, s1_np))
    hl = np.asarray(hidden_layers, np.float32)
    w0 = np.ascontiguousarray(hl[0].astype(mm_np))
    w1 = np.ascontiguousarray(hl[1].astype(mm_np))
    scT = np.ascontiguousarray(
        np.asarray(scorer, np.float32).reshape(KP, 128).T.astype(mm_np)
    )  # [128, 4]

    in_maps = []
    for c in range(NCORES):
        xb = x[c * BC: (c + 1) * BC]                       # [32, 256, 1024]
        xf = xb.reshape(ROWS, D)                            # [8192, 1024]
        xTc = np.ascontiguousarray(
            xf.reshape(NBLK, 512, KD, 128).transpose(0, 3, 2, 1).astype(s1_np)
        ).reshape(NBLK, 128, KD * 512)
        xpc = np.ascontiguousarray(xb[:, S - 2, :].T.astype(s1_np))  # [1024, 32]
        xcc = np.ascontiguousarray(xb[:, S - 1, :].T.astype(s1_np))  # [1024, 32]
        mb = mask[c * BC: (c + 1) * BC]                    # [32, 256]
        lm = np.full((BC, S), NEG, np.float32)
        lm[:, : S - 2][mb[:, : S - 2]] = 0.0
        in_maps.append(
            {
                "xT": xTc, "xprep": xpc, "xchild": xcc,
                "wh": wh, "wp": wp, "wc": wc, "w0": w0, "w1": w1,
                "scT": scT, "lmask": lm,
            }
        )
    return in_maps


_NC_CACHE = {}


def _get_nc(key="default"):
    if key not in _NC_CACHE:
        _NC_CACHE[key] = _build()
    return _NC_CACHE[key]


def kernel(x, proj_head, proj_prep, proj_child, hidden_layers, scorer, mask):
    in_maps = _host_prep(
        x, proj_head, proj_prep, proj_child, hidden_layers, scorer, mask
    )
    nc = _get_nc()
    res = bass_utils.run_bass_kernel_spmd(
        nc, in_maps, core_ids=list(range(NCORES))
    )
    out = np.concatenate([r["out"] for r in res.results], axis=0)
    return out.astype(np.float32)


if __name__ == "__main__":
    rng = np.random.default_rng(0)
    x = rng.standard_normal((B, S, D)).astype(np.float32)
    u = lambda shp: rng.uniform(-0.05, 0.05, shp).astype(np.float32)
    inputs = dict(
        x=x, proj_head=u((D, P)), proj_prep=u((D, P)), proj_child=u((D, P)),
        hidden_layers=u((2, P, P)), scorer=u((P,)),
        mask=rng.integers(0, 2, (B, S)).astype(bool),
    )
    out = kernel(**inputs)
    print("kernel out", out.shape, out.dtype, out[:2, :4])



# revision 2
# speedup vs baseline: 2.6673x; 2.6673x over previous
"""Trainium2 Bass kernel for nn_AttachmentPredictor.

Pipeline (per core, data-parallel over batch; 32 batches/core).

Sparsity: the reference zeroes every output where mask=0, so only
unmasked head positions (~50%) need scores at all.  The host packs each
batch's unmasked rows into a fixed-capacity slot of Q columns
(Q = max unmasked count over all batches, rounded up to 32; typically
160), giving 32*Q packed rows per core instead of 32*256.  Slot
boundaries are compile-time constants shared by all cores (SPMD), and
pad columns carry -1e9 in a log-mask so exp() kills them.

Per 512-row block:
  stage1: head projection, feature-major psum[jt] += Wh[dk,jt] @ xT[dk,:]
  tanh(Y1 + bias) -> c1, with per-(batch-slot x block) activation
  segments supplying the per-batch prep+child bias (all 32-aligned)
  stage2/3: hidden layers, feature-major, tanh -> c2, c3
  scorer: [1,512] psum rows of scores via M=1 matmuls
  epilogue: scores + logmask, exp per slot segment with accumulated
  per-slot partial sums.
Tail: combine partials, +EPS, reciprocal, scale packed exps, DMA the
packed [1, 32*Q] vector out; the host scatters to the full [B, S-2]
grid (zeros where masked).
"""

import ml_dtypes
import numpy as np

import concourse.bass as bass
import concourse.mybir as mybir
import concourse.tile as tile
from concourse import bass_utils
from concourse.bass import ts

F32 = mybir.dt.float32
F32R = mybir.dt.float32r
BF16 = mybir.dt.bfloat16
AF = mybir.ActivationFunctionType

B, S, D, P = 256, 256, 1024, 512
SH = S - 2                  # head positions per batch
NCORES = 8
BC = B // NCORES            # 32 batches per core
KD = D // 128               # 8 k-tiles over D
KP = P // 128               # 4 k-tiles over P
EPS = 1e-7
NEG = -1e9

OPTS = {
    "s1_dtype": "bf16",   # dtype for stage-1 x and Wh: "f32r" | "bf16"
    "mm_dtype": "bf16",   # dtype for stages 2/3, scorer, bias: "f32r" | "bf16"
    "xr_bufs": 4,
    "c_bufs": 10,
    "ps_bufs": 8,
}

_DT = {"f32r": F32R, "bf16": BF16, "f32": F32, "f16": mybir.dt.float16}
_NPDT = {"f32r": np.float32, "bf16": ml_dtypes.bfloat16, "f32": np.float32,
         "f16": np.float16}


def _slot_capacity(mask):
    """Slot width Q: max unmasked rows over all batches, rounded up to 32."""
    nb = np.asarray(mask)[:, :SH].sum(axis=1).max()
    q = max(32, int(-(-int(nb) // 32) * 32))
    return min(q, 256)


def _segments(q, nblk):
    """Per block: [(slot, lo, hi)] covering [0,512) by slot intersections."""
    segs = []
    for g in range(nblk):
        row = []
        for b in range(BC):
            lo = max(b * q, g * 512) - g * 512
            hi = min((b + 1) * q, (g + 1) * 512) - g * 512
            if lo < hi:
                row.append((b, lo, hi))
        segs.append(row)
    return segs


# ---------------------------------------------------------------------------
# walrus in this container accepts at most ONE sync wait per instruction;
# split extra waits onto preceding NoOps on the same engine.
def _split_waits(nc, maxw=1):
    ctr = 0
    for f in nc.m.functions:
        for blk in f.blocks:
            insts = blk.instructions
            newlist = []
            changed = False
            for inst in insts:
                si = inst.sync_info
                if si is not None and len(si.on_wait) > maxw:
                    waits = list(si.on_wait)
                    keep = waits[len(waits) - maxw:]
                    extra = waits[: len(waits) - maxw]
                    for j in range(0, len(extra), maxw):
                        ctr += 1
                        newlist.append(
                            mybir.InstNoOp(
                                name=f"waitsplit-{ctr}",
                                engine=inst.engine,
                                ins=[],
                                outs=[],
                                sync_info=mybir.SyncInfo(
                                    on_wait=extra[j: j + maxw], on_update=[]
                                ),
                            )
                        )
                    inst.sync_info = mybir.SyncInfo(
                        on_wait=keep, on_update=list(si.on_update)
                    )
                    changed = True
                newlist.append(inst)
            if changed:
                insts[:] = newlist


# ---------------------------------------------------------------------------
def _build(opts=None, reps=1, q=160):
    opts = dict(OPTS, **(opts or {}))
    nc = bass.Bass("TRN2", target_bir_lowering=False, debug=False)

    S1DT = _DT[opts["s1_dtype"]]
    MMDT = _DT[opts["mm_dtype"]]
    nblk = (BC * q) // 512
    assert (BC * q) % 512 == 0
    rows = nblk * 512
    segs = _segments(q, nblk)

    s1_dma = nc.sync
    mm_dma = nc.sync

    xT_d = nc.dram_tensor("xT", [nblk, 128, KD * 512], S1DT,
                          kind="ExternalInput").ap()
    xp_d = nc.dram_tensor("xprep", [D, BC], S1DT, kind="ExternalInput").ap()
    xc_d = nc.dram_tensor("xchild", [D, BC], S1DT, kind="ExternalInput").ap()
    wh_d = nc.dram_tensor("wh", [D, P], S1DT, kind="ExternalInput").ap()
    wp_d = nc.dram_tensor("wp", [D, P], S1DT, kind="ExternalInput").ap()
    wc_d = nc.dram_tensor("wc", [D, P], S1DT, kind="ExternalInput").ap()
    w0_d = nc.dram_tensor("w0", [P, P], MMDT, kind="ExternalInput").ap()
    w1_d = nc.dram_tensor("w1", [P, P], MMDT, kind="ExternalInput").ap()
    sc_d = nc.dram_tensor("scT", [128, KP], MMDT, kind="ExternalInput").ap()
    lm_d = nc.dram_tensor("lmask", [1, rows], F32, kind="ExternalInput").ap()
    out_d = nc.dram_tensor("out", [1, rows], F32, kind="ExternalOutput").ap()

    with tile.TileContext(nc) as tc:
        with (
            tc.tile_pool(name="consts", bufs=1) as consts,
            tc.tile_pool(name="ssb", bufs=3) as spool,
            tc.tile_pool(name="xr", bufs=opts["xr_bufs"]) as xpool,
            tc.tile_pool(name="acts", bufs=opts["c_bufs"]) as cpool,
            tc.tile_pool(name="ps", bufs=opts["ps_bufs"], space="PSUM") as pspool,
            tc.tile_pool(name="epi", bufs=1) as epi,
        ):
            # ---- constants -------------------------------------------------
            def load_packed(dram, k, n, dt, dma, tag):
                t = consts.tile([128, k * n], dt, tag=tag)
                dma.dma_start(
                    t[:].rearrange("p (k n) -> p k n", n=n),
                    dram.rearrange("(k p) n -> p k n", p=128),
                )
                return t

            wh_t = []
            for dk in range(KD):
                wt = consts.tile([128, P], S1DT, tag=f"wh{dk}", name=f"wh_t{dk}")
                s1_dma.dma_start(wt[:], wh_d[dk * 128: (dk + 1) * 128, :])
                wh_t.append(wt)
            xp_r = load_packed(xp_d, KD, BC, S1DT, mm_dma, "xp")
            xc_r = load_packed(xc_d, KD, BC, S1DT, mm_dma, "xc")
            sc_r = consts.tile([128, KP], MMDT, tag="sc")
            mm_dma.dma_start(sc_r[:], sc_d[:])
            lm_sb = consts.tile([1, rows], F32, tag="lm")
            mm_dma.dma_start(lm_sb[:], lm_d[:])

            # ---- per-batch bias, feature-major [128 j, 32 b] per j-tile ----
            psbs = [pspool.tile([128, BC], F32, tag="ps", name=f"psb_{jt}")
                    for jt in range(KP)]
            for i, (xs, w_d) in enumerate(((xp_r, wp_d), (xc_r, wc_d))):
                for dk in range(KD):
                    wst = spool.tile([128, P], S1DT, tag="wst")
                    mm_dma.dma_start(wst[:], w_d[dk * 128: (dk + 1) * 128, :])
                    for jt in range(KP):
                        nc.tensor.matmul(
                            psbs[jt][:],
                            wst[:, jt * 128: (jt + 1) * 128],
                            xs[:, dk * BC: (dk + 1) * BC],
                            start=(i == 0 and dk == 0),
                            stop=(i == 1 and dk == KD - 1),
                        )
            bias_fm = []
            for jt in range(KP):
                bf = consts.tile([128, BC], F32, tag=f"bias{jt}")
                nc.vector.tensor_copy(bf[:], psbs[jt][:])
                bias_fm.append(bf)

            w0_r = load_packed(w0_d, KP, P, MMDT, mm_dma, "w0")
            w1_r = load_packed(w1_d, KP, P, MMDT, mm_dma, "w1")

            exps_all = epi.tile([1, rows], F32, tag="exps")
            outv = epi.tile([1, rows], F32, tag="outv")
            parts = epi.tile([1, 2 * BC], F32, tag="parts")
            sums = epi.tile([1, BC], F32, tag="sums")
            recips = epi.tile([1, BC], F32, tag="recips")

            # ---- main loop -------------------------------------------------
            for _rep in range(reps):
                nc.gpsimd.memset(parts[:], 0.0)
                occ = {}
                for g in range(nblk):
                    xr = xpool.tile([128, KD * 512], S1DT, tag="xr")
                    hw = KD * 512 // 2
                    for h in range(2):
                        s1_dma.dma_start(
                            xr[:, h * hw: (h + 1) * hw],
                            xT_d[g, :, h * hw: (h + 1) * hw],
                        )

                    # stage 1 + per-batch bias via slot segments
                    c1 = [None] * KP
                    for jt in range(KP):
                        pss1 = pspool.tile([128, 512], F32, tag="ps",
                                           name=f"ps1_{g}_{jt}")
                        for dk in range(KD):
                            nc.tensor.matmul(
                                pss1[:],
                                wh_t[dk][:, jt * 128: (jt + 1) * 128],
                                xr[:, dk * 512: (dk + 1) * 512],
                                start=(dk == 0),
                                stop=(dk == KD - 1),
                            )
                        ct = cpool.tile([128, 512], MMDT, tag="c1")
                        for (b, lo, hi) in segs[g]:
                            nc.scalar.activation(
                                ct[:, lo:hi],
                                pss1[:, lo:hi],
                                AF.Tanh,
                                bias=bias_fm[jt][:, b: b + 1],
                            )
                        c1[jt] = ct

                    # stages 2, 3
                    c_in = c1
                    for stage, w_r in ((2, w0_r), (3, w1_r)):
                        c_out = [None] * KP
                        for qt in range(KP):
                            pss2 = pspool.tile([128, 512], F32, tag="ps",
                                               name=f"ps{stage}_{g}_{qt}")
                            for jk in range(KP):
                                nc.tensor.matmul(
                                    pss2[:],
                                    w_r[:, jk * P + qt * 128:
                                        jk * P + (qt + 1) * 128],
                                    c_in[jk][:],
                                    start=(jk == 0),
                                    stop=(jk == KP - 1),
                                )
                            ct = cpool.tile([128, 512], MMDT, tag=f"c{stage}")
                            nc.scalar.activation(ct[:], pss2[:], AF.Tanh)
                            c_out[qt] = ct
                        c_in = c_out

                    # scorer -> [1, 512] scores for this block
                    pss = pspool.tile([1, 512], F32, tag="ps",
                                      name=f"pssc_{g}")
                    for qk in range(KP):
                        nc.tensor.matmul(
                            pss[:],
                            sc_r[:, qk: qk + 1],
                            c_in[qk][:],
                            start=(qk == 0),
                            stop=(qk == KP - 1),
                        )
                    expin = spool.tile([1, 512], F32, tag="expin",
                                       name=f"ei_{g}")
                    nc.vector.tensor_add(
                        expin[:], pss[0:1, :],
                        lm_sb[0:1, g * 512: (g + 1) * 512],
                    )
                    for (b, lo, hi) in segs[g]:
                        k = occ.get(b, 0)
                        occ[b] = k + 1
                        assert k < 2
                        nc.scalar.activation(
                            exps_all[0:1, g * 512 + lo: g * 512 + hi],
                            expin[0:1, lo:hi],
                            AF.Exp,
                            accum_out=parts[0:1, k * BC + b: k * BC + b + 1],
                        )

                # ---- tail: combine partials, normalize, DMA out -----------
                nc.vector.tensor_add(sums[:], parts[0:1, 0:BC],
                                     parts[0:1, BC: 2 * BC])
                nc.vector.tensor_scalar_add(sums[:], sums[:], EPS)
                nc.vector.reciprocal(recips[:], sums[:])
                for g in range(nblk):
                    for (b, lo, hi) in segs[g]:
                        nc.vector.tensor_scalar_mul(
                            outv[0:1, g * 512 + lo: g * 512 + hi],
                            exps_all[0:1, g * 512 + lo: g * 512 + hi],
                            recips[0:1, b: b + 1],
                        )
                nc.sync.dma_start(out_d[:], outv[:])

    _split_waits(nc)
    return nc


# ---------------------------------------------------------------------------
class _Plan:
    def __init__(self, q, idx_lists):
        self.q = q
        self.nblk = (BC * q) // 512
        self.idx_lists = idx_lists  # [B] arrays of unmasked positions


def _host_prep(x, proj_head, proj_prep, proj_child, hidden_layers, scorer,
               mask, opts=None):
    opts = dict(OPTS, **(opts or {}))
    s1_np = _NPDT[opts["s1_dtype"]]
    mm_np = _NPDT[opts["mm_dtype"]]
    x = np.asarray(x, np.float32)
    mask = np.asarray(mask)
    q = _slot_capacity(mask)
    nblk = (BC * q) // 512
    rows = nblk * 512
    idx_lists = [np.nonzero(mask[b, :SH])[0] for b in range(B)]

    wh = np.ascontiguousarray(np.asarray(proj_head, s1_np))
    wp = np.ascontiguousarray(np.asarray(proj_prep, s1_np))
    wc = np.ascontiguousarray(np.asarray(proj_child, s1_np))
    hl = np.asarray(hidden_layers, np.float32)
    w0 = np.ascontiguousarray(hl[0].astype(mm_np))
    w1 = np.ascontiguousarray(hl[1].astype(mm_np))
    scT = np.ascontiguousarray(
        np.asarray(scorer, np.float32).reshape(KP, 128).T.astype(mm_np)
    )  # [128, 4]

    in_maps = []
    for c in range(NCORES):
        xb = x[c * BC: (c + 1) * BC]                        # [32, 256, 1024]
        packed = np.zeros((rows, D), np.float32)
        lm = np.full((1, rows), NEG, np.float32)
        for b in range(BC):
            idx = idx_lists[c * BC + b]
            n = len(idx)
            packed[b * q: b * q + n] = xb[b, idx, :]
            lm[0, b * q: b * q + n] = 0.0
        xTc = np.ascontiguousarray(
            packed.reshape(nblk, 512, KD, 128).transpose(0, 3, 2, 1)
            .astype(s1_np)
        ).reshape(nblk, 128, KD * 512)
        xpc = np.ascontiguousarray(xb[:, S - 2, :].T.astype(s1_np))  # [1024,32]
        xcc = np.ascontiguousarray(xb[:, S - 1, :].T.astype(s1_np))  # [1024,32]
        in_maps.append(
            {
                "xT": xTc, "xprep": xpc, "xchild": xcc,
                "wh": wh, "wp": wp, "wc": wc, "w0": w0, "w1": w1,
                "scT": scT, "lmask": lm,
            }
        )
    return in_maps, _Plan(q, idx_lists)


_NC_CACHE = {}


def _get_nc(q, opts=None, reps=1):
    key = (q, reps, tuple(sorted((opts or {}).items())))
    if key not in _NC_CACHE:
        _NC_CACHE[key] = _build(opts=opts, reps=reps, q=q)
    return _NC_CACHE[key]


def kernel(x, proj_head, proj_prep, proj_child, hidden_layers, scorer, mask):
    in_maps, plan = _host_prep(
        x, proj_head, proj_prep, proj_child, hidden_layers, scorer, mask
    )
    nc = _get_nc(plan.q)
    res = bass_utils.run_bass_kernel_spmd(
        nc, in_maps, core_ids=list(range(NCORES))
    )
    out = np.zeros((B, SH), np.float32)
    for c in range(NCORES):
        vals = res.results[c]["out"][0]
        for b in range(BC):
            idx = plan.idx_lists[c * BC + b]
            out[c * BC + b, idx] = vals[b * plan.q: b * plan.q + len(idx)]
    return out


if __name__ == "__main__":
    rng = np.random.default_rng(0)
    x = rng.standard_normal((B, S, D)).astype(np.float32)
    u = lambda shp: rng.uniform(-0.05, 0.05, shp).astype(np.float32)
    inputs = dict(
        x=x, proj_head=u((D, P)), proj_prep=u((D, P)), proj_child=u((D, P)),
        hidden_layers=u((2, P, P)), scorer=u((P,)),
        mask=rng.integers(0, 2, (B, S)).astype(bool),
    )
    out = kernel(**inputs)
    print("kernel out", out.shape, out.dtype, out[:2, :4])
